# revision 8
# baseline (speedup 1.0000x reference)
"""Trainium2 Bass kernel for nn_BitwiseCellularAutomata.

The reference runs a 100-step cellular automaton:
    h0 = bit_stimuli.int8         [B, N]
    W0 = identity                 [B, N, N]
    step: E = bit3(h); P = bit2(h)
          ce    = (W @ E > 0)
          E_new = E ^ ce
          P_new = P & ~E_new
          W    |= outer(E_new, ~P_new)
          h     = E_new<<3 | P_new<<2 | (h & 3)

Because W0 is the identity, at the very first step ce == E, so
E_new == E ^ E == 0 for every cell.  Then P_new == P, the wiring
outer-product is identically zero (E_new == 0), and W stays the
identity.  From step 2 on the state is a fixed point (E stays 0).
Hence for ANY input values and ANY steps >= 1:
    h_final = bit_stimuli & 7   (int8)     [bit 3 cleared, bits 2..0 kept]
    W_final = identity          (int8)
and for steps == 0: h = bit_stimuli.astype(int8), W = identity.
(Verified bit-exact against the reference scan output.)

Sharding: pure data parallel, batch 16 -> 2 per core across 8 cores,
no cross-device communication.

The runtime contract of run_bass_kernel_spmd zero-initializes
ExternalOutput buffers (native path: out_map = np.zeros handed to
run_neff; axon/PJRT path: zero buffers are passed as operands bound to
the output tensors and donated).  Kernels that don't write every
element rely on that, so W only needs its 2 x 16 diagonal [128,128]
blocks written.

Raw Bass (no TileContext): the Tile tail drain waits on one semaphore
per engine/DMA lane it saw, which exceeds this toolchain's
per-instruction sync-wait slot limit; with manual semaphores every
instruction carries at most one wait, and we also skip Tile's
tail barrier overhead.

Per core (cost-model ~7.2 us, dominated by fixed DMA latencies):
  * gpsimd builds T = I_128 (memset + one affine_select; iota j-p fits
    int8 range),
  * W block-diagonals: one strided DMA per batch, batch 0 on the sync
    HWDGE ring and batch 1 on the scalar HWDGE ring (parallel descriptor
    generation).  The source r-dim uses step 0 to replay the single
    I_128 tile for all 16 blocks; dest AP steps: partition 2048,
    block 128*2048+128, col 1.
  * h: the scalar ring loads the raw int32 input bytes first; DVE
    computes h = low_byte(x) & 7 from a byte view ((x & 0xFF) & 7 ==
    x & 7, no dtype cast needed); gpsimd's SWDGE stores h.
"""

import numpy as np

B, N = 16, 2048
N_CORES = 8
B_SHARD = B // N_CORES  # 2 batches per core
P = 128
RB = N // P  # 16 diagonal blocks per batch

_BUILD_CACHE = {}


def _build_bass(mask):
    """Build the per-core Bass module. mask=None means plain int8 cast."""
    import concourse.bass as bass
    import concourse.mybir as mybir

    nc = bass.Bass()
    x = nc.declare_dram_parameter(
        "bit_stimuli", [B_SHARD, N], mybir.dt.int32, isOutput=False
    )
    h_out = nc.declare_dram_parameter(
        "h_out", [B_SHARD, N], mybir.dt.int8, isOutput=True
    )
    w_out = nc.declare_dram_parameter(
        "W_out", [B_SHARD, N, N], mybir.dt.int8, isOutput=True
    )
    FB = B_SHARD * N * 4 // P  # bytes per partition of the raw int32 input
    w_handle = w_out[:].tensor

    with (
        nc.sbuf_tensor([P, FB], mybir.dt.int8) as xb,
        nc.sbuf_tensor([P, FB // 4], mybir.dt.int8) as ht,
        nc.sbuf_tensor([P, P], mybir.dt.int8) as T,
        nc.semaphore() as s_in,
        nc.semaphore() as s_T,
        nc.semaphore() as s_h,
        nc.semaphore() as s_out,
        nc.semaphore() as s_ho,
        nc.Block() as block,
    ):
        x_bytes = x[:].bitcast(mybir.dt.int8).flatten().rearrange("(p f) -> p f", p=P)
        h_dest = h_out[:].flatten().rearrange("(p f) -> p f", p=P)

        def w_diag_dma(engine, b):
            # dest element (p, r, j) -> W[b, r*128 + p, r*128 + j];
            # source replays T (step-0 r-dim).
            src = bass.AP(T[:].tensor, 0, [[P, P], [0, RB], [1, P]])
            dest = bass.AP(w_handle, b * N * N, [[N, P], [P * N + P, RB], [1, P]])
            engine.dma_start(out=dest, in_=src).then_inc(s_out, 16)

        @block.gpsimd
        def _(gpsimd):
            gpsimd.memset(T[:], 1).then_inc(s_T, 1)
            gpsimd.wait_ge(s_T, 1)
            # T[p, j] = 1 iff j == p; iota = j - p stays in int8 range.
            gpsimd.affine_select(
                T[:],
                T[:],
                pattern=[[1, P]],
                compare_op=mybir.AluOpType.is_equal,
                fill=0,
                base=0,
                channel_multiplier=-1,
            ).then_inc(s_T, 1)
            gpsimd.wait_ge(s_h, 1)
            gpsimd.dma_start(out=h_dest, in_=ht[:]).then_inc(s_ho, 16)

        @block.vector
        def _(vector):
            vector.wait_ge(s_in, 16)
            # low byte of each little-endian int32; (x & 0xFF) & 7 == x & 7
            x_lo = xb[:].rearrange("p (f four) -> p f four", four=4)[:, :, 0]
            if mask is None:
                vector.tensor_copy(ht[:], x_lo).then_inc(s_h, 1)
            else:
                vector.tensor_scalar(
                    ht[:], x_lo, mask, None, mybir.AluOpType.bitwise_and
                ).then_inc(s_h, 1)

        @block.scalar
        def _(scalar):
            scalar.dma_start(out=xb[:], in_=x_bytes).then_inc(s_in, 16)
            scalar.wait_ge(s_T, 2)
            w_diag_dma(scalar, 1)

        @block.sync
        def _(sync):
            sync.wait_ge(s_T, 2)
            w_diag_dma(sync, 0)
            sync.wait_ge(s_out, 32)
            sync.wait_ge(s_ho, 16)

    return nc


def _get_nc(mask):
    if mask not in _BUILD_CACHE:
        _BUILD_CACHE[mask] = _build_bass(mask)
    return _BUILD_CACHE[mask]


def run_sharded(bit_stimuli, steps, trace=False):
    """Shard, run on 8 cores, gather. Returns ((h, W), BassKernelResults)."""
    from concourse.bass_utils import run_bass_kernel_spmd

    bs = np.ascontiguousarray(np.asarray(bit_stimuli, dtype=np.int32))
    assert bs.shape == (B, N), bs.shape
    steps = int(np.asarray(steps))
    mask = None if steps == 0 else 7

    nc = _get_nc(mask)
    in_maps = [
        {"bit_stimuli": bs[i * B_SHARD : (i + 1) * B_SHARD]} for i in range(N_CORES)
    ]
    res = run_bass_kernel_spmd(nc, in_maps, core_ids=list(range(N_CORES)), trace=trace)

    h = np.concatenate(
        [np.asarray(r["h_out"], dtype=np.int8) for r in res.results], axis=0
    )
    W = np.concatenate(
        [np.asarray(r["W_out"], dtype=np.int8) for r in res.results], axis=0
    )
    return (h, W), res


def kernel(**inputs):
    (h, W), _ = run_sharded(inputs["bit_stimuli"], inputs["steps"])
    return h, W


# revision 9
# speedup vs baseline: 1.1604x; 1.1604x over previous
"""Trainium2 Bass kernel for nn_BitwiseCellularAutomata.

The reference runs a 100-step cellular automaton:
    h0 = bit_stimuli.int8         [B, N]
    W0 = identity                 [B, N, N]
    step: E = bit3(h); P = bit2(h)
          ce    = (W @ E > 0)
          E_new = E ^ ce
          P_new = P & ~E_new
          W    |= outer(E_new, ~P_new)
          h     = E_new<<3 | P_new<<2 | (h & 3)

Because W0 is the identity, at the very first step ce == E, so
E_new == E ^ E == 0 for every cell.  Then P_new == P, the wiring
outer-product is identically zero (E_new == 0), and W stays the
identity.  From step 2 on the state is a fixed point (E stays 0).
Hence for ANY input values and ANY steps >= 1:
    h_final = bit_stimuli & 7   (int8)     [bit 3 cleared, bits 2..0 kept]
    W_final = identity          (int8)
and for steps == 0: h = bit_stimuli.astype(int8), W = identity.
(Verified bit-exact against the reference scan output.)

Sharding: pure data parallel, batch 16 -> 2 per core across 8 cores,
no cross-device communication.

The runtime contract of run_bass_kernel_spmd zero-initializes
ExternalOutput buffers (native path: out_map = np.zeros handed to
run_neff; axon/PJRT path: zero buffers are passed as operands bound to
the output tensors and donated).  Kernels that don't write every
element rely on that, so W only needs its diagonal written: one
single-byte-descriptor scatter DMA per batch (2048 descriptors each,
at the cost model's per-descriptor floor), sourced from a [128, 16]
ones tile built by a single memset — no index generation needed at all.

Raw Bass (no TileContext): the Tile tail drain waits on one semaphore
per engine/DMA lane it saw, which exceeds this toolchain's
per-instruction sync-wait slot limit; with manual semaphores every
instruction carries at most one wait, and we also skip Tile's
tail barrier overhead.

Ring layout (cost model 6183 ns/core; the critical path is the
irreducible x-in -> DVE -> h-out fixed-latency chain, under which the
W scatter fully hides):
  * sync (qSP HWDGE): x-in, then h-out after DVE signals, then final
    completion waits — the h chain owns this ring end to end.
  * scalar (qAct HWDGE): both W diagonal scatters.
  * vector: memset of the ones tile (releases W immediately), then
    h = low_byte(x) & 7 from a byte view of the int32 input
    ((x & 0xFF) & 7 == x & 7, no dtype cast).
"""

import numpy as np

B, N = 16, 2048
N_CORES = 8
B_SHARD = B // N_CORES  # 2 batches per core
P = 128
RB = N // P  # 16 diagonal blocks per batch

_BUILD_CACHE = {}


def _build_bass(mask):
    """Build the per-core Bass module. mask=None means plain int8 cast."""
    import concourse.bass as bass
    import concourse.mybir as mybir

    nc = bass.Bass()
    x = nc.declare_dram_parameter(
        "bit_stimuli", [B_SHARD, N], mybir.dt.int32, isOutput=False
    )
    h_out = nc.declare_dram_parameter(
        "h_out", [B_SHARD, N], mybir.dt.int8, isOutput=True
    )
    w_out = nc.declare_dram_parameter(
        "W_out", [B_SHARD, N, N], mybir.dt.int8, isOutput=True
    )
    FB = B_SHARD * N * 4 // P  # bytes per partition of the raw int32 input
    w_handle = w_out[:].tensor

    with (
        nc.sbuf_tensor([P, FB], mybir.dt.int8) as xb,
        nc.sbuf_tensor([P, FB // 4], mybir.dt.int8) as ht,
        nc.sbuf_tensor([P, RB], mybir.dt.int8) as T16,
        nc.semaphore() as s_in,
        nc.semaphore() as s_T,
        nc.semaphore() as s_h,
        nc.semaphore() as s_out,
        nc.semaphore() as s_ho,
        nc.Block() as block,
    ):
        x_bytes = x[:].bitcast(mybir.dt.int8).flatten().rearrange("(p f) -> p f", p=P)
        h_dest = h_out[:].flatten().rearrange("(p f) -> p f", p=P)

        @block.vector
        def _(vector):
            vector.memset(T16[:], 1).then_inc(s_T, 1)
            vector.wait_ge(s_in, 16)
            # low byte of each little-endian int32; (x & 0xFF) & 7 == x & 7
            x_lo = xb[:].rearrange("p (f four) -> p f four", four=4)[:, :, 0]
            if mask is None:
                vector.tensor_copy(ht[:], x_lo).then_inc(s_h, 1)
            else:
                vector.tensor_scalar(
                    ht[:], x_lo, mask, None, mybir.AluOpType.bitwise_and
                ).then_inc(s_h, 1)

        @block.scalar
        def _(scalar):
            scalar.wait_ge(s_T, 1)
            for b in range(B_SHARD):
                # element (p, r) -> W[b, 128r + p, 128r + p] = 1
                src = bass.AP(T16[:].tensor, 0, [[RB, P], [1, RB], [1, 1]])
                dest = bass.AP(
                    w_handle, b * N * N, [[N + 1, P], [P * N + P, RB], [1, 1]]
                )
                with nc.allow_non_contiguous_dma(
                    reason="single-byte diagonal scatter is intended"
                ):
                    scalar.dma_start(out=dest, in_=src).then_inc(s_out, 16)

        @block.sync
        def _(sync):
            sync.dma_start(out=xb[:], in_=x_bytes).then_inc(s_in, 16)
            sync.wait_ge(s_h, 1)
            sync.dma_start(out=h_dest, in_=ht[:]).then_inc(s_ho, 16)
            sync.wait_ge(s_out, 32)
            sync.wait_ge(s_ho, 16)

    return nc


def _get_nc(mask):
    if mask not in _BUILD_CACHE:
        _BUILD_CACHE[mask] = _build_bass(mask)
    return _BUILD_CACHE[mask]


def run_sharded(bit_stimuli, steps, trace=False):
    """Shard, run on 8 cores, gather. Returns ((h, W), BassKernelResults)."""
    from concourse.bass_utils import run_bass_kernel_spmd

    bs = np.ascontiguousarray(np.asarray(bit_stimuli, dtype=np.int32))
    assert bs.shape == (B, N), bs.shape
    steps = int(np.asarray(steps))
    mask = None if steps == 0 else 7

    nc = _get_nc(mask)
    in_maps = [
        {"bit_stimuli": bs[i * B_SHARD : (i + 1) * B_SHARD]} for i in range(N_CORES)
    ]
    res = run_bass_kernel_spmd(nc, in_maps, core_ids=list(range(N_CORES)), trace=trace)

    h = np.concatenate(
        [np.asarray(r["h_out"], dtype=np.int8) for r in res.results], axis=0
    )
    W = np.concatenate(
        [np.asarray(r["W_out"], dtype=np.int8) for r in res.results], axis=0
    )
    return (h, W), res


def kernel(**inputs):
    (h, W), _ = run_sharded(inputs["bit_stimuli"], inputs["steps"])
    return h, W


# revision 12
# speedup vs baseline: 1.3165x; 1.1345x over previous
"""Trainium2 Bass kernel for nn_BitwiseCellularAutomata.

The reference runs a 100-step cellular automaton:
    h0 = bit_stimuli.int8         [B, N]
    W0 = identity                 [B, N, N]
    step: E = bit3(h); P = bit2(h)
          ce    = (W @ E > 0)
          E_new = E ^ ce
          P_new = P & ~E_new
          W    |= outer(E_new, ~P_new)
          h     = E_new<<3 | P_new<<2 | (h & 3)

Because W0 is the identity, at the very first step ce == E, so
E_new == E ^ E == 0 for every cell.  Then P_new == P, the wiring
outer-product is identically zero (E_new == 0), and W stays the
identity.  From step 2 on the state is a fixed point (E stays 0).
Hence for ANY input values and ANY steps >= 1:
    h_final = bit_stimuli & 7   (int8)     [bit 3 cleared, bits 2..0 kept]
    W_final = identity          (int8)
and for steps == 0: h = bit_stimuli.astype(int8), W = identity.
(Verified bit-exact against the reference scan output.)

Sharding: pure data parallel, batch 16 -> 2 per core across 8 cores,
no cross-device communication.

The runtime contract of run_bass_kernel_spmd zero-initializes
ExternalOutput buffers (native path: out_map = np.zeros handed to
run_neff; axon/PJRT path: zero buffers are passed as operands bound to
the output tensors and donated).  Kernels that don't write every
element rely on that, so W only needs its diagonal written: one
single-byte-descriptor scatter DMA per batch (2048 descriptors each,
at the cost model's per-descriptor floor), sourced from a [128, 16]
ones tile built by a single memset — no index generation needed at all.

Raw Bass (no TileContext): the Tile tail drain waits on one semaphore
per engine/DMA lane it saw, which exceeds this toolchain's
per-instruction sync-wait slot limit; with manual semaphores every
instruction carries at most one wait, and we also skip Tile's
tail barrier overhead.

Ring layout (cost model 6183 ns/core; the critical path is the
irreducible x-in -> DVE -> h-out fixed-latency chain, under which the
W scatter fully hides):
  * sync (qSP HWDGE): x-in, then h-out after DVE signals, then final
    completion waits — the h chain owns this ring end to end.
  * scalar (qAct HWDGE): both W diagonal scatters.
  * vector: memset of the ones tile (releases W immediately), then
    h = low_byte(x) & 7 from a byte view of the int32 input
    ((x & 0xFF) & 7 == x & 7, no dtype cast).
"""

import numpy as np

B, N = 16, 2048
N_CORES = 8
B_SHARD = B // N_CORES  # 2 batches per core
P = 128
RB = N // P  # 16 diagonal blocks per batch

_BUILD_CACHE = {}


def _strip_dead_preamble(nc):
    """Remove Bass-emitted preamble instructions this kernel provably never
    uses: the four const-tile memsets (walrus itself warns they have no
    reader) and the per-engine zero/broadcast register initializations for
    PE/Pool/SP/DVE (a BIR scan shows zero instructions read any register).
    The const memsets run serially on the Pool engine and gate the preamble
    all-engine barrier; SP's register moves delay its first DMA issue.
    Cost-model gain: ~730 ns (6183 -> 5450)."""
    for blk in nc.m.functions[0].blocks:
        il = blk.instructions
        for inst in list(il):
            tn = type(inst).__name__
            if tn == "InstMemset" and inst.outs and "const-" in str(inst.outs[0]):
                il.remove(inst)
            elif tn == "InstRegisterMove" and inst.outs:
                s = str(inst.outs[0])
                if any(
                    p in s for p in ("PE_", "Pool_", "SP_", "DVE_", "Activation_")
                ):
                    il.remove(inst)
    return nc


def _build_bass(mask):
    """Build the per-core Bass module. mask=None means plain int8 cast."""
    import concourse.bass as bass
    import concourse.mybir as mybir

    nc = bass.Bass()
    x = nc.declare_dram_parameter(
        "bit_stimuli", [B_SHARD, N], mybir.dt.int32, isOutput=False
    )
    h_out = nc.declare_dram_parameter(
        "h_out", [B_SHARD, N], mybir.dt.int8, isOutput=True
    )
    w_out = nc.declare_dram_parameter(
        "W_out", [B_SHARD, N, N], mybir.dt.int8, isOutput=True
    )
    FB = B_SHARD * N * 4 // P  # bytes per partition of the raw int32 input
    w_handle = w_out[:].tensor

    with (
        nc.sbuf_tensor([P, FB], mybir.dt.int8) as xb,
        nc.sbuf_tensor([P, FB // 4], mybir.dt.int8) as ht,
        nc.sbuf_tensor([P, RB], mybir.dt.int8) as T16,
        nc.semaphore() as s_in,
        nc.semaphore() as s_T,
        nc.semaphore() as s_h,
        nc.semaphore() as s_out,
        nc.semaphore() as s_ho,
        nc.Block() as block,
    ):
        x_bytes = x[:].bitcast(mybir.dt.int8).flatten().rearrange("(p f) -> p f", p=P)
        h_dest = h_out[:].flatten().rearrange("(p f) -> p f", p=P)

        @block.vector
        def _(vector):
            vector.memset(T16[:], 1).then_inc(s_T, 1)
            vector.wait_ge(s_in, 16)
            # low byte of each little-endian int32; (x & 0xFF) & 7 == x & 7
            x_lo = xb[:].rearrange("p (f four) -> p f four", four=4)[:, :, 0]
            if mask is None:
                vector.tensor_copy(ht[:], x_lo).then_inc(s_h, 1)
            else:
                vector.tensor_scalar(
                    ht[:], x_lo, mask, None, mybir.AluOpType.bitwise_and
                ).then_inc(s_h, 1)

        @block.scalar
        def _(scalar):
            scalar.wait_ge(s_T, 1)
            for b in range(B_SHARD):
                # element (p, r) -> W[b, 128r + p, 128r + p] = 1
                src = bass.AP(T16[:].tensor, 0, [[RB, P], [1, RB], [1, 1]])
                dest = bass.AP(
                    w_handle, b * N * N, [[N + 1, P], [P * N + P, RB], [1, 1]]
                )
                with nc.allow_non_contiguous_dma(
                    reason="single-byte diagonal scatter is intended"
                ):
                    scalar.dma_start(out=dest, in_=src).then_inc(s_out, 16)

        @block.sync
        def _(sync):
            sync.dma_start(out=xb[:], in_=x_bytes).then_inc(s_in, 16)
            sync.wait_ge(s_h, 1)
            sync.dma_start(out=h_dest, in_=ht[:]).then_inc(s_ho, 16)
            sync.wait_ge(s_out, 32)
            sync.wait_ge(s_ho, 16)

    return _strip_dead_preamble(nc)


def _get_nc(mask):
    if mask not in _BUILD_CACHE:
        _BUILD_CACHE[mask] = _build_bass(mask)
    return _BUILD_CACHE[mask]


def run_sharded(bit_stimuli, steps, trace=False):
    """Shard, run on 8 cores, gather. Returns ((h, W), BassKernelResults)."""
    from concourse.bass_utils import run_bass_kernel_spmd

    bs = np.ascontiguousarray(np.asarray(bit_stimuli, dtype=np.int32))
    assert bs.shape == (B, N), bs.shape
    steps = int(np.asarray(steps))
    mask = None if steps == 0 else 7

    nc = _get_nc(mask)
    in_maps = [
        {"bit_stimuli": bs[i * B_SHARD : (i + 1) * B_SHARD]} for i in range(N_CORES)
    ]
    res = run_bass_kernel_spmd(nc, in_maps, core_ids=list(range(N_CORES)), trace=trace)

    h = np.concatenate(
        [np.asarray(r["h_out"], dtype=np.int8) for r in res.results], axis=0
    )
    W = np.concatenate(
        [np.asarray(r["W_out"], dtype=np.int8) for r in res.results], axis=0
    )
    return (h, W), res


def kernel(**inputs):
    (h, W), _ = run_sharded(inputs["bit_stimuli"], inputs["steps"])
    return h, W


# revision 13
# speedup vs baseline: 1.4442x; 1.0970x over previous
"""Trainium2 Bass kernel for nn_BitwiseCellularAutomata.

The reference runs a 100-step cellular automaton:
    h0 = bit_stimuli.int8         [B, N]
    W0 = identity                 [B, N, N]
    step: E = bit3(h); P = bit2(h)
          ce    = (W @ E > 0)
          E_new = E ^ ce
          P_new = P & ~E_new
          W    |= outer(E_new, ~P_new)
          h     = E_new<<3 | P_new<<2 | (h & 3)

Because W0 is the identity, at the very first step ce == E, so
E_new == E ^ E == 0 for every cell.  Then P_new == P, the wiring
outer-product is identically zero (E_new == 0), and W stays the
identity.  From step 2 on the state is a fixed point (E stays 0).
Hence for ANY input values and ANY steps >= 1:
    h_final = bit_stimuli & 7   (int8)     [bit 3 cleared, bits 2..0 kept]
    W_final = identity          (int8)
and for steps == 0: h = bit_stimuli.astype(int8), W = identity.
(Verified bit-exact against the reference scan output.)

Sharding: pure data parallel, batch 16 -> 2 per core across 8 cores,
no cross-device communication.

The runtime contract of run_bass_kernel_spmd zero-initializes
ExternalOutput buffers (native path: out_map = np.zeros handed to
run_neff; axon/PJRT path: zero buffers are passed as operands bound to
the output tensors and donated).  Kernels that don't write every
element rely on that, so W only needs its diagonal written: one
single-byte-descriptor scatter DMA per batch (2048 descriptors each,
at the cost model's per-descriptor floor), sourced from a [128, 16]
ones tile built by a single memset — no index generation needed at all.

Raw Bass (no TileContext): the Tile tail drain waits on one semaphore
per engine/DMA lane it saw, which exceeds this toolchain's
per-instruction sync-wait slot limit; with manual semaphores every
instruction carries at most one wait, and we also skip Tile's
tail barrier overhead.

Ring layout (cost model 6183 ns/core; the critical path is the
irreducible x-in -> DVE -> h-out fixed-latency chain, under which the
W scatter fully hides):
  * sync (qSP HWDGE): x-in, then h-out after DVE signals, then final
    completion waits — the h chain owns this ring end to end.
  * scalar (qAct HWDGE): both W diagonal scatters.
  * vector: memset of the ones tile (releases W immediately), then
    h = low_byte(x) & 7 from a byte view of the int32 input
    ((x & 0xFF) & 7 == x & 7, no dtype cast).
"""

import numpy as np

B, N = 16, 2048
N_CORES = 8
B_SHARD = B // N_CORES  # 2 batches per core
P = 128
RB = N // P  # 16 diagonal blocks per batch

_BUILD_CACHE = {}


def _strip_dead_preamble(nc):
    """Remove Bass-emitted boilerplate this kernel provably never needs:

    * the four const-tile memsets (walrus itself warns they have no reader);
      they run serially on the Pool engine and gate the preamble barrier;
    * the per-engine zero/broadcast register initializations (a BIR scan
      shows zero instructions read any register); SP's moves delayed its
      first DMA issue by ~250 ns;
    * the preamble and exit all-engine barriers (drains + event-semaphore
      butterfly).  The drains carry no semaphore-reset duty
      (is_reset_sema=None); semaphore zeroing is runtime-level, which is
      also why repeated executions stay correct.  All user-visible ordering
      runs through explicit semaphores, and SP's final wait_ge on the
      output-DMA completion semaphores is the completion guarantee, so the
      barriers are pure ceremony here.  Verified by repeated-execution
      hardware soaks (bit-exact).

    Cost-model gain: ~1.2 us total (6183 -> 4968)."""
    blocks = list(nc.m.functions[0].blocks)
    for bi, blk in enumerate(blocks):
        il = blk.instructions
        barrier_block = bi == 0 or blk.name.endswith("_end")
        for inst in list(il):
            tn = type(inst).__name__
            if tn == "InstMemset" and inst.outs and "const-" in str(inst.outs[0]):
                il.remove(inst)
            elif tn == "InstRegisterMove" and inst.outs:
                s = str(inst.outs[0])
                if any(
                    p in s for p in ("PE_", "Pool_", "SP_", "DVE_", "Activation_")
                ):
                    il.remove(inst)
            elif barrier_block and tn in ("InstDrain", "InstEventSemaphore"):
                il.remove(inst)
    return nc


def _build_bass(mask):
    """Build the per-core Bass module. mask=None means plain int8 cast."""
    import concourse.bass as bass
    import concourse.mybir as mybir

    nc = bass.Bass()
    x = nc.declare_dram_parameter(
        "bit_stimuli", [B_SHARD, N], mybir.dt.int32, isOutput=False
    )
    h_out = nc.declare_dram_parameter(
        "h_out", [B_SHARD, N], mybir.dt.int8, isOutput=True
    )
    w_out = nc.declare_dram_parameter(
        "W_out", [B_SHARD, N, N], mybir.dt.int8, isOutput=True
    )
    FB = B_SHARD * N * 4 // P  # bytes per partition of the raw int32 input
    w_handle = w_out[:].tensor

    with (
        nc.sbuf_tensor([P, FB], mybir.dt.int8) as xb,
        nc.sbuf_tensor([P, FB // 4], mybir.dt.int8) as ht,
        nc.sbuf_tensor([P, RB], mybir.dt.int8) as T16,
        nc.semaphore() as s_in,
        nc.semaphore() as s_T,
        nc.semaphore() as s_h,
        nc.semaphore() as s_out,
        nc.semaphore() as s_ho,
        nc.Block() as block,
    ):
        x_bytes = x[:].bitcast(mybir.dt.int8).flatten().rearrange("(p f) -> p f", p=P)
        h_dest = h_out[:].flatten().rearrange("(p f) -> p f", p=P)

        @block.vector
        def _(vector):
            vector.memset(T16[:], 1).then_inc(s_T, 1)
            vector.wait_ge(s_in, 16)
            # low byte of each little-endian int32; (x & 0xFF) & 7 == x & 7
            x_lo = xb[:].rearrange("p (f four) -> p f four", four=4)[:, :, 0]
            if mask is None:
                vector.tensor_copy(ht[:], x_lo).then_inc(s_h, 1)
            else:
                vector.tensor_scalar(
                    ht[:], x_lo, mask, None, mybir.AluOpType.bitwise_and
                ).then_inc(s_h, 1)

        @block.scalar
        def _(scalar):
            scalar.wait_ge(s_T, 1)
            for b in range(B_SHARD):
                # element (p, r) -> W[b, 128r + p, 128r + p] = 1
                src = bass.AP(T16[:].tensor, 0, [[RB, P], [1, RB], [1, 1]])
                dest = bass.AP(
                    w_handle, b * N * N, [[N + 1, P], [P * N + P, RB], [1, 1]]
                )
                with nc.allow_non_contiguous_dma(
                    reason="single-byte diagonal scatter is intended"
                ):
                    scalar.dma_start(out=dest, in_=src).then_inc(s_out, 16)

        @block.sync
        def _(sync):
            sync.dma_start(out=xb[:], in_=x_bytes).then_inc(s_in, 16)
            sync.wait_ge(s_h, 1)
            sync.dma_start(out=h_dest, in_=ht[:]).then_inc(s_ho, 16)
            sync.wait_ge(s_out, 32)
            sync.wait_ge(s_ho, 16)

    return _strip_dead_preamble(nc)


def _get_nc(mask):
    if mask not in _BUILD_CACHE:
        _BUILD_CACHE[mask] = _build_bass(mask)
    return _BUILD_CACHE[mask]


def run_sharded(bit_stimuli, steps, trace=False):
    """Shard, run on 8 cores, gather. Returns ((h, W), BassKernelResults)."""
    from concourse.bass_utils import run_bass_kernel_spmd

    bs = np.ascontiguousarray(np.asarray(bit_stimuli, dtype=np.int32))
    assert bs.shape == (B, N), bs.shape
    steps = int(np.asarray(steps))
    mask = None if steps == 0 else 7

    nc = _get_nc(mask)
    in_maps = [
        {"bit_stimuli": bs[i * B_SHARD : (i + 1) * B_SHARD]} for i in range(N_CORES)
    ]
    res = run_bass_kernel_spmd(nc, in_maps, core_ids=list(range(N_CORES)), trace=trace)

    h = np.concatenate(
        [np.asarray(r["h_out"], dtype=np.int8) for r in res.results], axis=0
    )
    W = np.concatenate(
        [np.asarray(r["W_out"], dtype=np.int8) for r in res.results], axis=0
    )
    return (h, W), res


def kernel(**inputs):
    (h, W), _ = run_sharded(inputs["bit_stimuli"], inputs["steps"])
    return h, W
